# revision 43
# baseline (speedup 1.0000x reference)
"""Multi-head (per-task) 2-layer MLP classifier for Trainium2, 8 NeuronCores.

Strategy: expert-parallel with host-side dispatch. Rows of x are grouped by
task_id on the host (the all-to-all "dispatch"); core t gets all rows whose
task_id == t, zero-padded to a fixed PAD columns, pre-transposed to x^T
[D, PAD]. Each core then runs a dense 2-layer MLP for its own head only:

    H^T = relu(W1^T x^T + b1)        [H, PAD]   (psum: out=W1.T@xT, lhsT=W1)
    Y^T = W2^T H^T + b2              [C, PAD]   (lhsT=W2, rhs=H^T)

Everything stays "transposed" (feature dim on partitions, batch on the free
dim) so both matmuls chain without any on-device transpose, and both biases
are per-partition vectors. The host scatters Y^T columns back to the
original row order.

Schedule notes (from NTFF trace analysis; per-MM issue gaps measured on HW):
  - PSUM chunks stay 512-wide + a 16-wide runt (1040 -> 512/512/16). At
    N=512 matmuls stream at 216ns (= N/2.4GHz + NX, LDWEIGHTS hidden); at
    N<~450 the per-MM LDWEIGHTS stops hiding (N=347 measured 177ns, an
    0.51ns/col effective rate vs 0.42 at 512), and the N=16 runt matmuls
    stream at ~25-80ns, so equal-width chunking measured net slower.
  - ~26 N=128 warmup matmuls on a zeroed tile run while the first k-tiles
    are in flight (the PE would otherwise idle ~4us waiting for DMA): they
    flip the HAM clock gate (4/8 -> 8/8 after ~3.4us of *continuous* PE
    busy -- a 1us hole resets the window) so the real matmuls start at
    2.4GHz instead of paying ~2.7us of cold 1.2GHz issue gaps.
  - xt is split per k-tile into a chunk-0 piece (128KB, all eight queued
    first on the SP HWDGE ring) + remainder. Chunk 0's k-sweep is then
    PE-bound: per k it needs 128KB(xt)+256KB(w1) = ~1.1us of DMA vs the
    PE's 1.73us per k-group. Full 266KB xt k-tiles made it DMA-bound.
  - w1 k-tiles stream on the ACT HWDGE ring; w2 is pre-transposed on the
    host to [P, KH*P] so its DMA is contiguous 2KB lines (was a 1024x256B
    gather) and queues behind w1 -- it's only needed ~13us later at the
    first layer-2 matmul.
  - bias+relu for each m-group is split column-wise into a DVE half and an
    ACT half running in parallel (~400ns instead of ~740ns): a whole-width
    op per engine still paced the layer-2 k-sweep (and the PSUM-bank
    recycle the next chunk's layer-1 waits on) below the PE rate.
  - the runt's yt write (100 lines of 64B, per-packet-overhead-dominated)
    drains on the otherwise-idle ACT ring, concurrent with the chunk
    writes on the SP ring. yt itself is bf16 (KMM_YT32=1 for fp32):
    halves the output drain at the kernel tail for +~1e-3 rel err.
  - GpSimd carries only b1/b2 (it cannot touch PSUM); warmup count and the
    alternatives are env-tunable (KMM_WARM/KMM_CHUNK/KMM_RUNT_MID/...).

This computes each row through exactly one head (the reference computes all
8 heads and selects via one-hot -> 8x the FLOPs) and reads each expert's
weights from HBM exactly once across the whole chip.

Measured on 8xNC_v3 (max over cores, NTFF profile): ~51.8-52.0us in the
chip's un-throttled state vs ~55-57us for the v1 baseline, rel err ~4e-3
vs the fp32 reference with bf16 matmuls (fp32 PSUM accum). Of that, ~9us is the
fixed NEFF postamble (one semaphore-zero instruction per semaphore, split
across engines) plus ~2.5us of pre-stream DMA ramp; the matmul stream
itself runs at the bf16 PE roofline (~31us) + ~2us of DMA pacing. NOTE the
chip power-throttles under sustained load (PE 2.4 -> ~2.0GHz, HAM forced
cold): back-to-back measurements can read ~10us slower chip-wide.
KMM_DTYPE=f32r gives ~1.9e-4 (fp32r self-loads weights every matmul, ~70%
of bf16 PE throughput); f32 is exact-ish but 4x slower on the PE.
"""

import os

import numpy as np

import concourse.bacc as bacc
import concourse.bass as bass
import concourse.mybir as mybir
import concourse.tile as tile
from concourse.bass_utils import run_bass_kernel_spmd

# Problem constants (nn_MultiHeadClassifier: T tasks, 2-layer MLP heads)
T = 8          # tasks == cores
D = 1024       # d_model
HID = 1024     # hidden
C = 100        # classes
B = 8192       # batch
P = 128        # partitions
KD = D // P    # k-tiles in layer-1 contraction
KH = HID // P  # k-tiles in layer-2 contraction

# Per-core padded batch. Task counts for the graded inputs max out at 1040;
# _run grows this automatically if a different distribution needs more.
PAD_DEFAULT = 1040

_MM_DTYPES = {
    "f32": mybir.dt.float32,
    "f32r": mybir.dt.float32r,
    "bf16": mybir.dt.bfloat16,
}


def _chunks(total):
    """512-wide chunks + runt (PSUM bank = 512 fp32 per partition).

    Measured per-MM issue gaps: N=512 -> 216ns (streaming-bound, LDWEIGHTS
    hidden), N=347 -> 177ns (LDW-bound: the 116ns LDWEIGHTS + ~60ns commit
    doesn't hide below N~450), N=16 runt -> ~80ns. So max-width chunks win
    per column and the runt is cheap; near-equal 347-chunks measured net
    slower.
    """
    if os.environ.get("KMM_CHUNK", "512") == "equal":
        n = max(1, -(-total // 512))
        base, rem = divmod(total, n)
        out, o = [], 0
        for i in range(n):
            c = base + (1 if i < rem else 0)
            out.append((o, c))
            o += c
        return out
    out, o = [], 0
    while o < total:
        c = min(512, total - o)
        out.append((o, c))
        o += c
    return out


def build_program(pad, mm_dtype="bf16", n_warm=26):
    """One SPMD NeuronCore program: dense 2-layer MLP on [D, pad] x^T."""
    mm_dt = _MM_DTYPES[mm_dtype]
    f32 = mybir.dt.float32
    # Tensors consumed by the matmuls carry the matmul dtype end-to-end
    # (walrus requires fp32r-consumed buffers to be *produced* as fp32r).
    io_dt = mm_dt

    def mm(ap):
        return ap.bitcast(mm_dt) if ap.dtype != mm_dt else ap

    nc = bacc.Bacc()
    xt = nc.dram_tensor("xt", [D, pad], io_dt, kind="ExternalInput")
    w1 = nc.dram_tensor("w1", [D, HID], io_dt, kind="ExternalInput")
    b1 = nc.dram_tensor("b1", [P, KH], f32, kind="ExternalInput")
    w2t = nc.dram_tensor("w2t", [P, KH * P], io_dt, kind="ExternalInput")
    b2 = nc.dram_tensor("b2", [C, 1], f32, kind="ExternalInput")
    # yt in bf16 halves the output-drain bytes; quantizing the final logits
    # adds <=2e-3 to the ~3e-3 bf16-matmul error (gate is 2e-2).
    yt_dt = f32 if os.environ.get("KMM_YT32", "0") == "1" else mybir.dt.bfloat16
    yt = nc.dram_tensor("yt", [C, pad], yt_dt, kind="ExternalOutput")

    w1_t = w1.rearrange("(k p) h -> k p h", p=P)
    xt_t = xt.rearrange("(k p) b -> k p b", p=P)
    chunks = _chunks(pad)
    # Process the runt chunk second-to-last-in-size but NOT last in time: its
    # yt slice is 64B lines (100 packets of 16*4B), ~4us of drain on the DMA
    # queue. In the middle it hides under the next chunk's compute and the
    # kernel ends on a dense 2KB-line write instead.
    if (len(chunks) > 2 and chunks[-1][1] < chunks[0][1]
            and os.environ.get("KMM_RUNT_MID", "0") == "1"):
        chunks = [chunks[0], chunks[-1]] + chunks[1:-1]

    with tile.TileContext(nc) as tc:
        with (
            tc.tile_pool(name="weights", bufs=1) as wpool,
            tc.tile_pool(name="acts", bufs=1) as apool,
            tc.tile_pool(name="ps", bufs=8, space="PSUM") as pspool,
            tc.tile_pool(name="outs", bufs=3) as opool,
        ):
            # PE warmup while the first k-tiles are still in flight: no data
            # deps, N=128 cold matmuls issue every ~107-128ns, so n_warm=26
            # keeps the PE busy ~3us -- enough to trip the HAM un-throttle
            # right as real data lands. All write the same psum tile
            # (in-order PE, no reader, WAW is free).
            if n_warm:
                warm = wpool.tile([P, P], io_dt, name="warm", tag="warm")
                # memset on gpsimd: its queue is free right after the
                # framework const-memsets (~6.2us), ~1us before the DVE
                # gets there -- warmups (and so the HAM un-throttle at
                # start+3.4-6.8us, phase-dependent) begin that much sooner.
                nc.gpsimd.memset(warm[:], 0.0)
                pw = pspool.tile([P, 512], f32, name="ps_w", tag="ps")
                for _ in range(n_warm):
                    nc.tensor.matmul(
                        out=pw[:, 0:P], lhsT=warm[:], rhs=warm[:],
                        start=True, stop=True,
                    )

            # Small early tensors on the SWDGE (gpsimd) queue.
            b1_sb = wpool.tile([P, KH], f32, name="b1", tag="b1")
            nc.gpsimd.dma_start(out=b1_sb[:], in_=b1[:])
            b2_sb = wpool.tile([C, 1], f32, name="b2", tag="b2")
            nc.gpsimd.dma_start(out=b2_sb[:], in_=b2[:])

            # Bulk streams: xt on the SP HWDGE ring, split per k-tile into a
            # chunk-0 piece (queued first) + remainder (queued after all the
            # chunk-0 pieces). Chunk 0's k-sweep is then PE-bound: its per-k
            # DMA demand is 128KB(xt)+256KB(w1) = 1.07us < the PE's 1.73us
            # per k-group, whereas full 266KB k-tiles made it DMA-bound.
            # w1 k-tiles stream on the ACT HWDGE ring; w2 queues behind w1
            # (ACT ring) -- needed only ~13us later, at the first L2 mm.
            c0w = chunks[0][1]
            # w1_6 rides the SP ring after the chunk-0 xt pieces (lands
            # ~15us vs its 20.8us deadline): 8 w1 tiles on the ACT ring
            # alone undershoot the PE's 1.73us/k chunk-0 cadence on cores
            # with below-par HBM share (measured 1.5us k6/k7 stall).
            w1_sync = {int(s) for s in os.environ.get("KMM_W1SYNC", "6").split(",")
                       if s} if KD > 4 else set()
            xt_a, xt_b, w1_sb = [], [], []
            for k in range(KD):
                xa = wpool.tile([P, c0w], io_dt, name=f"xta_{k}", tag=f"xta_{k}")
                nc.sync.dma_start(out=xa[:], in_=xt_t[k, :, 0:c0w])
                xt_a.append(xa)
                wk = wpool.tile([P, HID], io_dt, name=f"w1_{k}", tag=f"w1_{k}")
                if k == 0 and os.environ.get("KMM_W1K0SPLIT", "0") == "1":
                    # k0 in halves: the first 4 m-group matmuls need only
                    # cols 0:512, so compute starts ~0.7us earlier.
                    nc.scalar.dma_start(out=wk[:, 0:HID // 2],
                                        in_=w1_t[k, :, 0:HID // 2])
                    nc.scalar.dma_start(out=wk[:, HID // 2:HID],
                                        in_=w1_t[k, :, HID // 2:HID])
                elif k not in w1_sync:
                    nc.scalar.dma_start(out=wk[:], in_=w1_t[k, :, :])
                w1_sb.append(wk)
            for k in sorted(w1_sync):
                nc.sync.dma_start(out=w1_sb[k][:], in_=w1_t[k, :, :])
            for k in range(KD):
                xb = wpool.tile([P, pad - c0w], io_dt, name=f"xtb_{k}",
                                tag=f"xtb_{k}")
                nc.sync.dma_start(out=xb[:], in_=xt_t[k, :, c0w:pad])
                xt_b.append(xb)
            w2_sb = wpool.tile([P, KH * P], io_dt, name="w2", tag="w2")
            nc.scalar.dma_start(out=w2_sb[:], in_=w2t[:])

            def xt_slice(k, o, cw):
                if o + cw <= c0w:
                    return xt_a[k][:, o:o + cw]
                return xt_b[k][:, o - c0w:o - c0w + cw]

            h_sb = [apool.tile([P, pad], io_dt, name=f"h_{m}", tag=f"h_{m}")
                    for m in range(KH)]

            relu = mybir.ActivationFunctionType.Relu
            for ci, (o, cw) in enumerate(chunks):
                # layer 1: all KH h-tile groups resident in PSUM, k swept in
                # the middle so PE consumes (w1_k, xt_k) right as each DMA
                # lands instead of stalling a single group on the last tile.
                pss = [pspool.tile([P, 512], f32, name=f"ps_{m}", tag="ps")
                       for m in range(KH)]
                for k in range(KD):
                    for m in range(KH):
                        nc.tensor.matmul(
                            out=pss[m][:, :cw],
                            lhsT=mm(w1_sb[k][:, m * P:(m + 1) * P]),
                            rhs=mm(xt_slice(k, o, cw)),
                            start=(k == 0),
                            stop=(k == KD - 1),
                        )
                # bias+relu: each m-group is split column-wise into a DVE
                # half and an ACT half running in parallel, so relu(m)
                # completes ~400ns after its k7 matmul instead of ~740ns.
                # One whole-width op per engine alternating still paced the
                # L2 k-sweep (and the PSUM-bank recycle the next chunk's L1
                # waits on) at ~390ns/matmul vs the PE's 216ns. (GpSimd
                # cannot read PSUM -- walrus birverifier rejects it.)
                # (tiny runt chunks: splitting a [128,16] op loses to its
                # fixed overhead -- alternate whole ops between the engines
                # instead, halving the per-m cadence the runt L2 waits on)
                half = (cw + 1) // 2 if cw >= 128 else cw
                for m in range(KH):
                    if cw > half:
                        nc.vector.tensor_scalar(
                            out=h_sb[m][:, o:o + half],
                            in0=pss[m][:, :half],
                            scalar1=b1_sb[:, m:m + 1],
                            scalar2=0.0,
                            op0=mybir.AluOpType.add,
                            op1=mybir.AluOpType.max,
                        )
                        nc.scalar.activation(
                            h_sb[m][:, o + half:o + cw],
                            pss[m][:, half:cw],
                            relu,
                            bias=b1_sb[:, m:m + 1],
                        )
                    elif m % 2 == 0:
                        nc.vector.tensor_scalar(
                            out=h_sb[m][:, o:o + cw],
                            in0=pss[m][:, :cw],
                            scalar1=b1_sb[:, m:m + 1],
                            scalar2=0.0,
                            op0=mybir.AluOpType.add,
                            op1=mybir.AluOpType.max,
                        )
                    else:
                        nc.scalar.activation(
                            h_sb[m][:, o:o + cw],
                            pss[m][:, :cw],
                            relu,
                            bias=b1_sb[:, m:m + 1],
                        )
                # layer 2: Y^T chunk = sum_k W2[k].T @ H^T[k] + b2
                ps2 = pspool.tile([P, 512], f32, name="ps2", tag="ps")
                for k in range(KH):
                    nc.tensor.matmul(
                        out=ps2[:, :cw],
                        lhsT=mm(w2_sb[:, k * P:(k + 1) * P]),
                        rhs=mm(h_sb[k][:, o:o + cw]),
                        start=(k == 0),
                        stop=(k == KH - 1),
                    )
                ot = opool.tile([P, 512], yt_dt, name="ot", tag="ot")
                nc.vector.tensor_scalar_add(
                    out=ot[:C, :cw],
                    in0=ps2[:C, :cw],
                    scalar1=b2_sb[:, 0:1],
                )
                # The runt's yt slice is 64B lines (16 cols x 4B) -- ~15ns
                # of payload but ~200ns of per-packet overhead x100. Drain
                # it on the otherwise-idle ACT ring so it runs concurrently
                # with the big 2KB-line chunk writes on the SP ring.
                yt_eng = nc.scalar if cw < 128 else nc.sync
                yt_eng.dma_start(out=yt[:, o:o + cw], in_=ot[:C, :cw])
    return nc


def _pad_cols(a, n):
    out = np.zeros((a.shape[0], n), dtype=a.dtype)
    out[:, :a.shape[1]] = a
    return out


def _route(task_id):
    """Group rows by task. Returns (row-index list per task, counts)."""
    task_id = np.asarray(task_id)
    order = np.argsort(task_id, kind="stable")
    counts = np.bincount(task_id.astype(np.int64), minlength=T)
    offs = np.zeros(T + 1, dtype=np.int64)
    np.cumsum(counts, out=offs[1:])
    rows = [order[offs[t]:offs[t + 1]] for t in range(T)]
    return rows, counts


def _run(inputs, trace=False):
    x = np.ascontiguousarray(np.asarray(inputs["x"], dtype=np.float32))
    task_id = np.asarray(inputs["task_id"])
    W1 = np.asarray(inputs["W1"], dtype=np.float32)
    b1 = np.asarray(inputs["b1"], dtype=np.float32)
    W2 = np.asarray(inputs["W2"], dtype=np.float32)
    b2 = np.asarray(inputs["b2"], dtype=np.float32)

    mm_dtype = os.environ.get("KMM_DTYPE", "bf16")
    n_warm = int(os.environ.get("KMM_WARM", "26"))
    pad = int(os.environ.get("KMM_PAD", PAD_DEFAULT))
    rows, counts = _route(task_id)
    if counts.max() > pad:  # unexpected distribution: grow pad to fit
        pad = int(-(-int(counts.max()) // 16) * 16)

    io_np = np.float32
    if mm_dtype == "bf16":
        import ml_dtypes
        io_np = ml_dtypes.bfloat16

    in_maps = []
    for t in range(T):
        xt = np.zeros((D, pad), dtype=io_np)
        xt[:, :counts[t]] = x[rows[t]].T
        # w2 pre-transposed to [P, KH*P]: w2t[p, k*P + c] = W2pad[k*P+p, c]
        # so the on-device DMA is fully contiguous 2KB lines and the L2
        # lhsT slice for k-tile k is w2_sb[:, k*P:(k+1)*P].
        w2p = _pad_cols(W2[t], P)
        w2t = np.ascontiguousarray(
            w2p.reshape(KH, P, P).transpose(1, 0, 2).reshape(P, KH * P)
        )
        in_maps.append({
            "xt": xt,
            "w1": np.ascontiguousarray(W1[t]).astype(io_np),
            "b1": np.ascontiguousarray(b1[t].reshape(KH, P).T.astype(np.float32)),
            "w2t": w2t.astype(io_np),
            "b2": np.ascontiguousarray(b2[t][:, None].astype(np.float32)),
        })

    nc = build_program(pad, mm_dtype, n_warm)
    nc.finalize()  # Bacc passes: legalize sync waits (<=1 per instruction)
    res = run_bass_kernel_spmd(
        nc, in_maps, core_ids=list(range(T)), trace=trace,
        trace_cores=list(range(T)) if trace else None,
        tmpdir=os.environ.get("KMM_TMPDIR"),
    )

    out = np.empty((task_id.shape[0], C), dtype=np.float32)
    for t in range(T):
        yt_res = np.asarray(res.results[t]["yt"]).astype(np.float32)
        out[rows[t]] = yt_res[:, :counts[t]].T
    return out, res


def kernel(**inputs):
    out, _ = _run(inputs, trace=False)
    return out


# revision 45
# speedup vs baseline: 1.0150x; 1.0150x over previous
"""Multi-head (per-task) 2-layer MLP classifier for Trainium2, 8 NeuronCores.

Strategy: expert-parallel with host-side dispatch. Rows of x are grouped by
task_id on the host (the all-to-all "dispatch"); core t gets all rows whose
task_id == t, zero-padded to a fixed PAD columns, pre-transposed to x^T
[D, PAD]. Each core then runs a dense 2-layer MLP for its own head only:

    H^T = relu(W1^T x^T + b1)        [H, PAD]   (psum: out=W1.T@xT, lhsT=W1)
    Y^T = W2^T H^T + b2              [C, PAD]   (lhsT=W2, rhs=H^T)

Everything stays "transposed" (feature dim on partitions, batch on the free
dim) so both matmuls chain without any on-device transpose, and both biases
are per-partition vectors. The host scatters Y^T columns back to the
original row order.

Schedule notes (from NTFF trace analysis; per-MM issue gaps measured on HW):
  - PSUM chunks stay 512-wide + a 16-wide runt (1040 -> 512/512/16). At
    N=512 matmuls stream at 216ns (= N/2.4GHz + NX, LDWEIGHTS hidden); at
    N<~450 the per-MM LDWEIGHTS stops hiding (N=347 measured 177ns, an
    0.51ns/col effective rate vs 0.42 at 512), and the N=16 runt matmuls
    stream at ~25-80ns, so equal-width chunking measured net slower.
  - ~26 N=128 warmup matmuls on a zeroed tile run while the first k-tiles
    are in flight (the PE would otherwise idle ~4us waiting for DMA): they
    flip the HAM clock gate (4/8 -> 8/8 after ~3.4us of *continuous* PE
    busy -- a 1us hole resets the window) so the real matmuls start at
    2.4GHz instead of paying ~2.7us of cold 1.2GHz issue gaps.
  - xt is split per k-tile into a chunk-0 piece (128KB, all eight queued
    first on the SP HWDGE ring) + remainder. Chunk 0's k-sweep is then
    PE-bound: per k it needs 128KB(xt)+256KB(w1) = ~1.1us of DMA vs the
    PE's 1.73us per k-group. Full 266KB xt k-tiles made it DMA-bound.
  - w1 k-tiles stream on the ACT HWDGE ring; w2 is pre-transposed on the
    host to [P, KH*P] so its DMA is contiguous 2KB lines (was a 1024x256B
    gather) and queues behind w1 -- it's only needed ~13us later at the
    first layer-2 matmul.
  - bias+relu for each m-group is split column-wise into a DVE half and an
    ACT half running in parallel (~400ns instead of ~740ns): a whole-width
    op per engine still paced the layer-2 k-sweep (and the PSUM-bank
    recycle the next chunk's layer-1 waits on) below the PE rate.
  - the runt's yt write (100 lines of 64B, per-packet-overhead-dominated)
    drains on the otherwise-idle ACT ring, concurrent with the chunk
    writes on the SP ring. yt itself is bf16 (KMM_YT32=1 for fp32):
    halves the output drain at the kernel tail for +~1e-3 rel err.
  - GpSimd carries only b1/b2 (it cannot touch PSUM); warmup count and the
    alternatives are env-tunable (KMM_WARM/KMM_CHUNK/KMM_RUNT_MID/...).

This computes each row through exactly one head (the reference computes all
8 heads and selects via one-hot -> 8x the FLOPs) and reads each expert's
weights from HBM exactly once across the whole chip.

Measured on 8xNC_v3 (max over cores, NTFF profile): ~51.8-52.0us in the
chip's un-throttled state vs ~55-57us for the v1 baseline, rel err ~4e-3
vs the fp32 reference with bf16 matmuls (fp32 PSUM accum). Of that, ~9us is the
fixed NEFF postamble (one semaphore-zero instruction per semaphore, split
across engines) plus ~2.5us of pre-stream DMA ramp; the matmul stream
itself runs at the bf16 PE roofline (~31us) + ~2us of DMA pacing. NOTE the
chip power-throttles under sustained load (PE 2.4 -> ~2.0GHz, HAM forced
cold): back-to-back measurements can read ~10us slower chip-wide.
KMM_DTYPE=f32r gives ~1.9e-4 (fp32r self-loads weights every matmul, ~70%
of bf16 PE throughput); f32 is exact-ish but 4x slower on the PE.
"""

import os

import numpy as np

import concourse.bacc as bacc
import concourse.bass as bass
import concourse.mybir as mybir
import concourse.tile as tile
from concourse.bass_utils import run_bass_kernel_spmd

# Problem constants (nn_MultiHeadClassifier: T tasks, 2-layer MLP heads)
T = 8          # tasks == cores
D = 1024       # d_model
HID = 1024     # hidden
C = 100        # classes
B = 8192       # batch
P = 128        # partitions
KD = D // P    # k-tiles in layer-1 contraction
KH = HID // P  # k-tiles in layer-2 contraction

# Per-core padded batch. Task counts for the graded inputs max out at 1040;
# _run grows this automatically if a different distribution needs more.
PAD_DEFAULT = 1040

_MM_DTYPES = {
    "f32": mybir.dt.float32,
    "f32r": mybir.dt.float32r,
    "bf16": mybir.dt.bfloat16,
}


def _chunks(total):
    """512-wide chunks + runt (PSUM bank = 512 fp32 per partition).

    Measured per-MM issue gaps: N=512 -> 216ns (streaming-bound, LDWEIGHTS
    hidden), N=347 -> 177ns (LDW-bound: the 116ns LDWEIGHTS + ~60ns commit
    doesn't hide below N~450), N=16 runt -> ~80ns. So max-width chunks win
    per column and the runt is cheap; near-equal 347-chunks measured net
    slower.
    """
    if os.environ.get("KMM_CHUNK", "512") == "equal":
        n = max(1, -(-total // 512))
        base, rem = divmod(total, n)
        out, o = [], 0
        for i in range(n):
            c = base + (1 if i < rem else 0)
            out.append((o, c))
            o += c
        return out
    out, o = [], 0
    while o < total:
        c = min(512, total - o)
        out.append((o, c))
        o += c
    return out


def build_program(pad, mm_dtype="bf16", n_warm=26):
    """One SPMD NeuronCore program: dense 2-layer MLP on [D, pad] x^T."""
    mm_dt = _MM_DTYPES[mm_dtype]
    f32 = mybir.dt.float32
    # Tensors consumed by the matmuls carry the matmul dtype end-to-end
    # (walrus requires fp32r-consumed buffers to be *produced* as fp32r).
    io_dt = mm_dt

    def mm(ap):
        return ap.bitcast(mm_dt) if ap.dtype != mm_dt else ap

    nc = bacc.Bacc()
    xt = nc.dram_tensor("xt", [D, pad], io_dt, kind="ExternalInput")
    w1 = nc.dram_tensor("w1", [D, HID], io_dt, kind="ExternalInput")
    b1 = nc.dram_tensor("b1", [P, KH], f32, kind="ExternalInput")
    w2t = nc.dram_tensor("w2t", [P, KH * P], io_dt, kind="ExternalInput")
    b2 = nc.dram_tensor("b2", [C, 1], f32, kind="ExternalInput")
    # yt in bf16 halves the output-drain bytes; quantizing the final logits
    # adds <=2e-3 to the ~3e-3 bf16-matmul error (gate is 2e-2).
    yt_dt = f32 if os.environ.get("KMM_YT32", "0") == "1" else mybir.dt.bfloat16
    yt = nc.dram_tensor("yt", [C, pad], yt_dt, kind="ExternalOutput")

    w1_t = w1.rearrange("(k p) h -> k p h", p=P)
    xt_t = xt.rearrange("(k p) b -> k p b", p=P)
    chunks = _chunks(pad)
    # Process the runt chunk second-to-last-in-size but NOT last in time: its
    # yt slice is 64B lines (100 packets of 16*4B), ~4us of drain on the DMA
    # queue. In the middle it hides under the next chunk's compute and the
    # kernel ends on a dense 2KB-line write instead.
    if (len(chunks) > 2 and chunks[-1][1] < chunks[0][1]
            and os.environ.get("KMM_RUNT_MID", "0") == "1"):
        chunks = [chunks[0], chunks[-1]] + chunks[1:-1]

    with tile.TileContext(nc) as tc:
        with (
            tc.tile_pool(name="weights", bufs=1) as wpool,
            tc.tile_pool(name="acts", bufs=1) as apool,
            tc.tile_pool(name="ps", bufs=8, space="PSUM") as pspool,
            tc.tile_pool(name="outs", bufs=3) as opool,
        ):
            # PE warmup while the first k-tiles are still in flight: no data
            # deps, N=128 cold matmuls issue every ~107-128ns, so n_warm=26
            # keeps the PE busy ~3us -- enough to trip the HAM un-throttle
            # right as real data lands. All write the same psum tile
            # (in-order PE, no reader, WAW is free).
            if n_warm:
                warm = wpool.tile([P, P], io_dt, name="warm", tag="warm")
                nc.vector.memset(warm[:], 0.0)
                pw = pspool.tile([P, 512], f32, name="ps_w", tag="ps")
                for _ in range(n_warm):
                    nc.tensor.matmul(
                        out=pw[:, 0:P], lhsT=warm[:], rhs=warm[:],
                        start=True, stop=True,
                    )

            # Small early tensors on the SWDGE (gpsimd) queue.
            b1_sb = wpool.tile([P, KH], f32, name="b1", tag="b1")
            nc.gpsimd.dma_start(out=b1_sb[:], in_=b1[:])
            b2_sb = wpool.tile([C, 1], f32, name="b2", tag="b2")
            nc.gpsimd.dma_start(out=b2_sb[:], in_=b2[:])

            # Bulk streams: xt on the SP HWDGE ring, split per k-tile into a
            # chunk-0 piece (queued first) + remainder (queued after all the
            # chunk-0 pieces). Chunk 0's k-sweep is then PE-bound: its per-k
            # DMA demand is 128KB(xt)+256KB(w1) = 1.07us < the PE's 1.73us
            # per k-group, whereas full 266KB k-tiles made it DMA-bound.
            # w1 k-tiles stream on the ACT HWDGE ring; w2 queues behind w1
            # (ACT ring) -- needed only ~13us later, at the first L2 mm.
            c0w = chunks[0][1]
            # w1_6 rides the SP ring after the chunk-0 xt pieces (lands
            # ~15us vs its 20.8us deadline): 8 w1 tiles on the ACT ring
            # alone undershoot the PE's 1.73us/k chunk-0 cadence on cores
            # with below-par HBM share (measured 1.5us k6/k7 stall).
            w1_sync = {int(s) for s in os.environ.get("KMM_W1SYNC", "6").split(",")
                       if s} if KD > 4 else set()
            xt_a, xt_b, w1_sb = [], [], []
            for k in range(KD):
                xa = wpool.tile([P, c0w], io_dt, name=f"xta_{k}", tag=f"xta_{k}")
                nc.sync.dma_start(out=xa[:], in_=xt_t[k, :, 0:c0w])
                xt_a.append(xa)
                wk = wpool.tile([P, HID], io_dt, name=f"w1_{k}", tag=f"w1_{k}")
                if k == 0 and os.environ.get("KMM_W1K0SPLIT", "0") == "1":
                    # k0 in halves: the first 4 m-group matmuls need only
                    # cols 0:512, so compute starts ~0.7us earlier.
                    nc.scalar.dma_start(out=wk[:, 0:HID // 2],
                                        in_=w1_t[k, :, 0:HID // 2])
                    nc.scalar.dma_start(out=wk[:, HID // 2:HID],
                                        in_=w1_t[k, :, HID // 2:HID])
                elif k not in w1_sync:
                    nc.scalar.dma_start(out=wk[:], in_=w1_t[k, :, :])
                w1_sb.append(wk)
            for k in sorted(w1_sync):
                nc.sync.dma_start(out=w1_sb[k][:], in_=w1_t[k, :, :])
            for k in range(KD):
                xb = wpool.tile([P, pad - c0w], io_dt, name=f"xtb_{k}",
                                tag=f"xtb_{k}")
                nc.sync.dma_start(out=xb[:], in_=xt_t[k, :, c0w:pad])
                xt_b.append(xb)
            w2_sb = wpool.tile([P, KH * P], io_dt, name="w2", tag="w2")
            nc.scalar.dma_start(out=w2_sb[:], in_=w2t[:])

            def xt_slice(k, o, cw):
                if o + cw <= c0w:
                    return xt_a[k][:, o:o + cw]
                return xt_b[k][:, o - c0w:o - c0w + cw]

            h_sb = [apool.tile([P, pad], io_dt, name=f"h_{m}", tag=f"h_{m}")
                    for m in range(KH)]

            # Merge the outputs of every chunk past the first into ONE SBUF
            # slab written by a single dense DMA: the runt's own yt slice is
            # 32B lines whose per-packet overhead (~1.3us for 6KB) binds the
            # kernel tail; merged into c1's columns it rides 1056B lines.
            merge_out = (os.environ.get("KMM_MERGE_OUT", "1") == "1"
                         and len(chunks) == 3
                         and chunks[1][0] == chunks[0][1]
                         and chunks[2][0] == chunks[1][0] + chunks[1][1])
            if merge_out:
                mo_o = chunks[1][0]
                mo_w = pad - mo_o
                omerge = wpool.tile([P, mo_w], yt_dt, name="om", tag="om")

            relu = mybir.ActivationFunctionType.Relu
            for ci, (o, cw) in enumerate(chunks):
                # layer 1: all KH h-tile groups resident in PSUM, k swept in
                # the middle so PE consumes (w1_k, xt_k) right as each DMA
                # lands instead of stalling a single group on the last tile.
                pss = [pspool.tile([P, 512], f32, name=f"ps_{m}", tag="ps")
                       for m in range(KH)]
                for k in range(KD):
                    for m in range(KH):
                        nc.tensor.matmul(
                            out=pss[m][:, :cw],
                            lhsT=mm(w1_sb[k][:, m * P:(m + 1) * P]),
                            rhs=mm(xt_slice(k, o, cw)),
                            start=(k == 0),
                            stop=(k == KD - 1),
                        )
                # bias+relu: each m-group is split column-wise into a DVE
                # half and an ACT half running in parallel, so relu(m)
                # completes ~400ns after its k7 matmul instead of ~740ns.
                # One whole-width op per engine alternating still paced the
                # L2 k-sweep (and the PSUM-bank recycle the next chunk's L1
                # waits on) at ~390ns/matmul vs the PE's 216ns. (GpSimd
                # cannot read PSUM -- walrus birverifier rejects it.)
                # (tiny runt chunks: splitting a [128,16] op loses to its
                # fixed overhead -- alternate whole ops between the engines
                # instead, halving the per-m cadence the runt L2 waits on)
                half = (cw + 1) // 2 if cw >= 128 else cw
                for m in range(KH):
                    if cw > half:
                        nc.vector.tensor_scalar(
                            out=h_sb[m][:, o:o + half],
                            in0=pss[m][:, :half],
                            scalar1=b1_sb[:, m:m + 1],
                            scalar2=0.0,
                            op0=mybir.AluOpType.add,
                            op1=mybir.AluOpType.max,
                        )
                        nc.scalar.activation(
                            h_sb[m][:, o + half:o + cw],
                            pss[m][:, half:cw],
                            relu,
                            bias=b1_sb[:, m:m + 1],
                        )
                    elif m % 2 == 0:
                        nc.vector.tensor_scalar(
                            out=h_sb[m][:, o:o + cw],
                            in0=pss[m][:, :cw],
                            scalar1=b1_sb[:, m:m + 1],
                            scalar2=0.0,
                            op0=mybir.AluOpType.add,
                            op1=mybir.AluOpType.max,
                        )
                    else:
                        nc.scalar.activation(
                            h_sb[m][:, o:o + cw],
                            pss[m][:, :cw],
                            relu,
                            bias=b1_sb[:, m:m + 1],
                        )
                # layer 2: Y^T chunk = sum_k W2[k].T @ H^T[k] + b2
                ps2 = pspool.tile([P, 512], f32, name="ps2", tag="ps")
                for k in range(KH):
                    nc.tensor.matmul(
                        out=ps2[:, :cw],
                        lhsT=mm(w2_sb[:, k * P:(k + 1) * P]),
                        rhs=mm(h_sb[k][:, o:o + cw]),
                        start=(k == 0),
                        stop=(k == KH - 1),
                    )
                if merge_out and o >= mo_o:
                    nc.vector.tensor_scalar_add(
                        out=omerge[:C, o - mo_o:o - mo_o + cw],
                        in0=ps2[:C, :cw],
                        scalar1=b2_sb[:, 0:1],
                    )
                    if o + cw == pad:
                        nc.sync.dma_start(out=yt[:, mo_o:pad],
                                          in_=omerge[:C, :mo_w])
                    continue
                ot = opool.tile([P, 512], yt_dt, name="ot", tag="ot")
                nc.vector.tensor_scalar_add(
                    out=ot[:C, :cw],
                    in0=ps2[:C, :cw],
                    scalar1=b2_sb[:, 0:1],
                )
                # Non-merged fallback: the runt's tiny-line yt write drains
                # on the otherwise-idle ACT ring, concurrent with the dense
                # chunk writes on the SP ring.
                yt_eng = nc.scalar if cw < 128 else nc.sync
                yt_eng.dma_start(out=yt[:, o:o + cw], in_=ot[:C, :cw])
    return nc


def _pad_cols(a, n):
    out = np.zeros((a.shape[0], n), dtype=a.dtype)
    out[:, :a.shape[1]] = a
    return out


def _route(task_id):
    """Group rows by task. Returns (row-index list per task, counts)."""
    task_id = np.asarray(task_id)
    order = np.argsort(task_id, kind="stable")
    counts = np.bincount(task_id.astype(np.int64), minlength=T)
    offs = np.zeros(T + 1, dtype=np.int64)
    np.cumsum(counts, out=offs[1:])
    rows = [order[offs[t]:offs[t + 1]] for t in range(T)]
    return rows, counts


def _run(inputs, trace=False):
    x = np.ascontiguousarray(np.asarray(inputs["x"], dtype=np.float32))
    task_id = np.asarray(inputs["task_id"])
    W1 = np.asarray(inputs["W1"], dtype=np.float32)
    b1 = np.asarray(inputs["b1"], dtype=np.float32)
    W2 = np.asarray(inputs["W2"], dtype=np.float32)
    b2 = np.asarray(inputs["b2"], dtype=np.float32)

    mm_dtype = os.environ.get("KMM_DTYPE", "bf16")
    n_warm = int(os.environ.get("KMM_WARM", "26"))
    pad = int(os.environ.get("KMM_PAD", PAD_DEFAULT))
    rows, counts = _route(task_id)
    if counts.max() > pad:  # unexpected distribution: grow pad to fit
        pad = int(-(-int(counts.max()) // 16) * 16)

    io_np = np.float32
    if mm_dtype == "bf16":
        import ml_dtypes
        io_np = ml_dtypes.bfloat16

    in_maps = []
    for t in range(T):
        xt = np.zeros((D, pad), dtype=io_np)
        xt[:, :counts[t]] = x[rows[t]].T
        # w2 pre-transposed to [P, KH*P]: w2t[p, k*P + c] = W2pad[k*P+p, c]
        # so the on-device DMA is fully contiguous 2KB lines and the L2
        # lhsT slice for k-tile k is w2_sb[:, k*P:(k+1)*P].
        w2p = _pad_cols(W2[t], P)
        w2t = np.ascontiguousarray(
            w2p.reshape(KH, P, P).transpose(1, 0, 2).reshape(P, KH * P)
        )
        in_maps.append({
            "xt": xt,
            "w1": np.ascontiguousarray(W1[t]).astype(io_np),
            "b1": np.ascontiguousarray(b1[t].reshape(KH, P).T.astype(np.float32)),
            "w2t": w2t.astype(io_np),
            "b2": np.ascontiguousarray(b2[t][:, None].astype(np.float32)),
        })

    nc = build_program(pad, mm_dtype, n_warm)
    nc.finalize()  # Bacc passes: legalize sync waits (<=1 per instruction)
    res = run_bass_kernel_spmd(
        nc, in_maps, core_ids=list(range(T)), trace=trace,
        trace_cores=list(range(T)) if trace else None,
        tmpdir=os.environ.get("KMM_TMPDIR"),
    )

    out = np.empty((task_id.shape[0], C), dtype=np.float32)
    for t in range(T):
        yt_res = np.asarray(res.results[t]["yt"]).astype(np.float32)
        out[rows[t]] = yt_res[:, :counts[t]].T
    return out, res


def kernel(**inputs):
    out, _ = _run(inputs, trace=False)
    return out
